# revision 1
# baseline (speedup 1.0000x reference)
"""TRN2 Bass kernel for nn_HTModel: hierarchical-Tucker tensor network forward.

Per-core dataflow (pure data parallel over batch, BC=512 rows/core):
  X [BC,64,64] --PE-transpose--> X^T tiles [s,b] --MLP (W1,W2,W3aug)-->
  h3aug [33,b] --leaf (Q0=W4@P0 fold)--> t0 [64,b] --binary-counter tree
  fold (P1..P5, eager over j)--> pair [512,b] --flipped top matmul
  (pair stationary, Ptop moving)--> out [b,1000].

All matmuls run with operands bitcast to float32r (1 cycle/row on the PE at
free>=256 vs 4 for fp32; HW relerr ~1.5e-4 per contraction, probed).
"""
import sys

sys.path.insert(0, '/opt/trn_rl_repo')

import functools
import numpy as np
from contextlib import ExitStack

import concourse.bacc as bacc
import concourse.tile as tile
from concourse import mybir

F32 = mybir.dt.float32
F32R = mybir.dt.float32r
AFT = mybir.ActivationFunctionType

N_CORES = 8
B, NJ, S, M, Y = 4096, 64, 64, 32, 1000
RNK = [64, 128, 256, 512, 512, 512]   # rank of level-l tree output, l=0 leaf
BC = B // N_CORES                      # 512 batch rows per core
PI = [1, 1, 2, 4, 4, 4]                # partition-tiles per level (RNK/128)
USE_F32R = True


def _body(nc, tc, T):
    ctx = ExitStack()
    with ctx:
        wp = ctx.enter_context(tc.tile_pool(name="wp", bufs=1))
        ws = ctx.enter_context(tc.tile_pool(name="ws", bufs=2))
        xp = ctx.enter_context(tc.tile_pool(name="xp", bufs=2))
        hp = ctx.enter_context(tc.tile_pool(name="hp", bufs=2))
        pp = ctx.enter_context(tc.tile_pool(name="pp", bufs=2))
        tp = ctx.enter_context(tc.tile_pool(name="tp", bufs=1))
        op = ctx.enter_context(tc.tile_pool(name="op", bufs=1))
        ps = ctx.enter_context(tc.tile_pool(name="ps", bufs=8, space="PSUM"))

        # resident weights
        ident = wp.tile([128, 128], F32R)
        nc.sync.dma_start(ident[:], T["ident"][:])
        w1 = wp.tile([64, 128], F32R)
        nc.sync.dma_start(w1[:], T["w1"][:])
        w2 = wp.tile([128, 64], F32R)
        nc.sync.dma_start(w2[:], T["w2"][:])
        w3a = wp.tile([64, 33], F32R)
        nc.sync.dma_start(w3a[:], T["w3a"][:])
        b1v = wp.tile([128, 1], F32)
        nc.sync.dma_start(b1v[:], T["b1v"][:])
        b2v = wp.tile([64, 1], F32)
        nc.sync.dma_start(b2v[:], T["b2v"][:])
        b3v = wp.tile([33, 1], F32)
        nc.sync.dma_start(b3v[:], T["b3v"][:])
        ptop = wp.tile([128, 4000], F32R)   # (pt, yh, 500)
        nc.sync.dma_start(ptop[:], T["ptopm"][:])

        pend = []
        for f_, fw in enumerate((512, 512, 1024, 2048, 2048, 2048)):
            pd = tp.tile([64 if f_ == 0 else 128, fw], F32R,
                         tag=f"pend{f_}", name=f"pend{f_}")
            pend.append(pd)
        pairT = tp.tile([128, 2048], F32R, tag="pairT")
        stream = {}

        def fold(f, i, tiles):
            """Combine even child pend[f] with odd child `tiles` (psum) at
            level f, contract with P_{f+1}[i] -> level f+1 child psum tiles."""
            if f == 5:
                for p in range(4):
                    nc.vector.tensor_mul(
                        pairT[:, p * 512:(p + 1) * 512],
                        pend[5][:, p * 512:(p + 1) * 512], tiles[p][:])
                return []
            rin, rout = RNK[f], RNK[f + 1]
            pi_in, no = PI[f], (RNK[f + 1] + 127) // 128
            prow = min(128, rin)
            tag = f"prod{f}"
            pool = pp if f < 2 else tp
            pr = pool.tile([prow, pi_in * 512], F32R, tag=tag)
            for p in range(pi_in):
                pz = min(128, rin - p * 128)
                nc.vector.tensor_mul(
                    pr[:pz, p * 512:(p + 1) * 512],
                    pend[f][:pz, p * 512:(p + 1) * 512], tiles[p][:])
            wlen = pi_in * no * 128
            if f == 0:      # 32 folds; quarter blob = 8 folds
                if i % 8 == 0:
                    w_ = ws.tile([64, 1024], F32R, tag="p1q", name="p1q")
                    nc.sync.dma_start(
                        w_[:], T["p1"][:, (i // 8) * 1024:(i // 8 + 1) * 1024])
                    stream["p1"] = w_
                wt = stream["p1"][:, (i % 8) * wlen:(i % 8 + 1) * wlen]
            elif f == 1:    # 16 folds; quarter blob = 4 folds
                if i % 4 == 0:
                    w_ = ws.tile([128, 1024], F32R, tag="p2q", name="p2q")
                    nc.sync.dma_start(
                        w_[:], T["p2"][:, (i // 4) * 1024:(i // 4 + 1) * 1024])
                    stream["p2"] = w_
                wt = stream["p2"][:, (i % 4) * wlen:(i % 4 + 1) * wlen]
            else:
                wtile = ws.tile([prow, wlen], F32R, tag=f"w{f + 1}s",
                                name="wts", bufs=(2 if f == 2 else 1))
                nc.sync.dma_start(
                    wtile[:], T[f"p{f + 1}"][:, i * wlen:(i + 1) * wlen])
                wt = wtile[:]
            outs = []
            for ot in range(no):
                osz = min(128, rout - ot * 128)
                o = ps.tile([osz, 512], F32, tag="ps")
                for p in range(pi_in):
                    pz = min(128, rin - p * 128)
                    nc.tensor.matmul(
                        o[:],
                        (wt[:pz, p * (no * 128) + ot * 128:
                              p * (no * 128) + ot * 128 + osz]),
                        (pr[:pz, p * 512:(p + 1) * 512]),
                        start=(p == 0), stop=(p == pi_in - 1))
                outs.append(o)
            return outs

        xchunks = [None] * 4
        xTe = xTo = None
        for j in range(NJ):
            jp, sub = j // 2, j % 2
            if j % 16 == 0:
                q0q = ws.tile([33, 1024], F32R, tag="q0q", name="q0q")
                nc.sync.dma_start(
                    q0q[:], T["q0"][:, (j // 16) * 1024:(j // 16 + 1) * 1024])
                stream["q0"] = q0q
                for bb in range(4):
                    xc = xp.tile([128, 1024], F32R, tag=f"xc{bb}", name="xc")
                    nc.gpsimd.dma_start(
                        xc[:],
                        T["x"][bb * 128:(bb + 1) * 128,
                               j:j + 16, :].rearrange("b j s -> b (j s)"))
                    xchunks[bb] = xc
            if sub == 0:
                xTe = hp.tile([64, 512], F32R, tag="xTe", name="xTe")
                xTo = hp.tile([64, 512], F32R, tag="xTo", name="xTo")
                pst = ps.tile([128, 512], F32R, tag="ps", name="pst")
                for bb in range(4):
                    nc.tensor.transpose(
                        pst[:, bb * 128:(bb + 1) * 128],
                        xchunks[bb][:, (jp % 8) * 128:(jp % 8 + 1) * 128],
                        ident[:])
                nc.vector.tensor_copy(xTe[:], pst[0:64, :])
                nc.vector.tensor_copy(xTo[:], pst[64:128, :])
            if True:
                xt = xTe if sub == 0 else xTo
                ps1 = ps.tile([128, 512], F32, tag="ps")
                nc.tensor.matmul(ps1[:], (w1[:]), (xt[:]), start=True, stop=True)
                h1 = hp.tile([128, 512], F32R, tag="h1")
                nc.vector.tensor_scalar(h1[:], ps1[:], b1v[:], 0.0,
                                        mybir.AluOpType.add,
                                        mybir.AluOpType.max)
                ps2 = ps.tile([64, 512], F32, tag="ps")
                nc.tensor.matmul(ps2[:], (w2[:]), (h1[:]), start=True, stop=True)
                h2 = hp.tile([64, 512], F32R, tag="h2")
                nc.scalar.activation(h2[:], ps2[:], AFT.Relu, bias=b2v[:])
                ps3 = ps.tile([33, 512], F32, tag="ps")
                nc.tensor.matmul(ps3[:], (w3a[:]), (h2[:]), start=True, stop=True)
                h3 = hp.tile([33, 512], F32R, tag="h3")
                nc.scalar.activation(h3[:], ps3[:], AFT.Relu, bias=b3v[:])
                t0 = ps.tile([64, 512], F32, tag="ps")
                nc.tensor.matmul(
                    t0[:],
                    (stream["q0"][:, (j % 16) * 64:(j % 16 + 1) * 64]),
                    (h3[:]), start=True, stop=True)
                # binary-counter eager tree fold
                tiles, f, c = [t0], 0, j
                while c % 2 == 1 and f < 6:
                    tiles = fold(f, j >> (f + 1), tiles)
                    f += 1
                    c //= 2
                if f < 6:
                    for p, tl in enumerate(tiles):
                        pz = min(128, RNK[f] - p * 128)
                        nc.scalar.copy(
                            pend[f][:pz, p * 512:(p + 1) * 512], tl[:])

        # top: out[b, y] = sum_a pair[a, b] * Ptop[y, a], pair stationary
        for bt in range(4):
            outb = op.tile([128, 1000], F32, tag="outb")
            for yh in range(2):
                pt_ps = ps.tile([128, 500], F32, tag="ps")
                for pt in range(4):
                    nc.tensor.matmul(
                        pt_ps[:],
                        (pairT[:, pt * 512 + bt * 128:pt * 512 + bt * 128 + 128]),
                        (ptop[:, (pt * 2 + yh) * 500:(pt * 2 + yh + 1) * 500]),
                        start=(pt == 0), stop=(pt == 3))
                nc.scalar.copy(outb[:, yh * 500:(yh + 1) * 500], pt_ps[:])
            nc.sync.dma_start(T["out"][bt * 128:(bt + 1) * 128, :], outb[:])


def build_nc(reps=1):
    nc = bacc.Bacc()
    T = {}
    T["x"] = nc.declare_dram_parameter("x", [BC, NJ, S], F32R, isOutput=False)
    T["w1"] = nc.declare_dram_parameter("w1", [64, 128], F32R, isOutput=False)
    T["w2"] = nc.declare_dram_parameter("w2", [128, 64], F32R, isOutput=False)
    T["w3a"] = nc.declare_dram_parameter("w3a", [64, 33], F32R, isOutput=False)
    T["b1v"] = nc.declare_dram_parameter("b1v", [128, 1], F32, isOutput=False)
    T["b2v"] = nc.declare_dram_parameter("b2v", [64, 1], F32, isOutput=False)
    T["b3v"] = nc.declare_dram_parameter("b3v", [33, 1], F32, isOutput=False)
    T["q0"] = nc.declare_dram_parameter("q0", [33, NJ * 64], F32R, isOutput=False)
    T["p1"] = nc.declare_dram_parameter("p1", [64, 32 * 128], F32R, isOutput=False)
    T["p2"] = nc.declare_dram_parameter("p2", [128, 16 * 2 * 128], F32R, isOutput=False)
    T["p3"] = nc.declare_dram_parameter("p3", [128, 8 * 2 * 4 * 128], F32R, isOutput=False)
    T["p4"] = nc.declare_dram_parameter("p4", [128, 4 * 4 * 4 * 128], F32R, isOutput=False)
    T["p5"] = nc.declare_dram_parameter("p5", [128, 2 * 4 * 4 * 128], F32R, isOutput=False)
    T["ptopm"] = nc.declare_dram_parameter("ptopm", [128, 4000], F32R, isOutput=False)
    T["ident"] = nc.declare_dram_parameter("ident", [128, 128], F32R, isOutput=False)
    T["out"] = nc.declare_dram_parameter("out", [BC, Y], F32, isOutput=True)
    with tile.TileContext(nc) as tc:
        for _ in range(reps):
            _body(nc, tc, T)
    nc.compile()
    return nc


def _tree_blob(P):
    """P (nj, r_out, r_in) -> lhsT blob [min(128,r_in), nj*pi*no*128]."""
    nj, r_out, r_in = P.shape
    pi, no = (r_in + 127) // 128, (r_out + 127) // 128
    psz = min(128, r_in)
    W = np.transpose(P, (0, 2, 1)).astype(np.float64)      # (nj, r_in, r_out)
    W = W.reshape(nj, pi, psz, no, min(128, r_out))
    W = np.transpose(W, (2, 0, 1, 3, 4)).reshape(psz, -1)
    return np.ascontiguousarray(W.astype(np.float32))


def prepack(inputs):
    f = {k: np.asarray(v, dtype=np.float64) for k, v in inputs.items()
         if k != "X"}
    blobs = {}
    blobs["w1"] = np.ascontiguousarray(f["W1"].astype(np.float32))
    blobs["w2"] = np.ascontiguousarray(f["W2"].astype(np.float32))
    w3a = np.zeros((64, 33), np.float64)
    w3a[:, :32] = f["W3"]
    blobs["w3a"] = np.ascontiguousarray(w3a.astype(np.float32))
    blobs["b1v"] = np.ascontiguousarray(f["b1"].reshape(128, 1).astype(np.float32))
    blobs["b2v"] = np.ascontiguousarray(f["b2"].reshape(64, 1).astype(np.float32))
    b3v = np.concatenate([f["b3"], [1.0]]).reshape(33, 1)
    blobs["b3v"] = np.ascontiguousarray(b3v.astype(np.float32))
    # leaf: fold W4 (and b4) into P0:  t0[a] = sum_k h3[k] Q0[k,a] + c0[a]
    q0 = np.einsum("km,jam->jka", f["W4"], f["P0"])         # (nj, 32, 64)
    c0 = np.einsum("jam,m->ja", f["P0"], f["b4"])           # (nj, 64)
    q0a = np.concatenate([q0, c0[:, None, :]], axis=1)      # (nj, 33, 64)
    blobs["q0"] = np.ascontiguousarray(
        np.transpose(q0a, (1, 0, 2)).reshape(33, -1).astype(np.float32))
    for l, nm in ((1, "p1"), (2, "p2"), (3, "p3"), (4, "p4"), (5, "p5")):
        blobs[nm] = _tree_blob(np.asarray(inputs[f"P{l}"], np.float64))
    ptop = f["Ptop"]                                        # (1000, 512)
    A = ptop.T.reshape(4, 128, 2, 500)                      # [pt, part, yh, yy]
    blobs["ptopm"] = np.ascontiguousarray(
        np.transpose(A, (1, 0, 2, 3)).reshape(128, 4000).astype(np.float32))
    blobs["ident"] = np.eye(128, dtype=np.float32)
    return blobs


@functools.lru_cache(maxsize=2)
def _cached_nc(reps=1):
    return build_nc(reps)


def kernel(**inputs):
    from concourse.bass_utils import run_bass_kernel_spmd
    nc = _cached_nc(1)
    blobs = prepack(inputs)
    X = np.ascontiguousarray(np.asarray(inputs["X"], np.float32))
    in_maps = [dict(blobs, x=X[c * BC:(c + 1) * BC]) for c in range(N_CORES)]
    res = run_bass_kernel_spmd(nc, in_maps, list(range(N_CORES)))
    return np.concatenate([res.results[c]["out"] for c in range(N_CORES)], axis=0)

